# revision 19
# baseline (speedup 1.0000x reference)
"""Single-head causal self-attention on 8 TRN2 NeuronCores.

Problem: x [8, 4096, 1024] f32, Wq/Wk/Wv [1024, 128] f32
  q/k/v = x @ W*;  out = softmax(causal(q k^T / sqrt(128))) @ v   -> [8, 4096, 128] f32

Sharding: data-parallel over batch B=8 -> one batch element per core, weights
replicated. No collectives needed.

Per-core plan (T=4096, C=1024, D=128), bf16 matmul inputs / f32 PSUM:
  - host supplies x^T packed as 32 contiguous [128, 1024] tiles (t-chunk major)
    and weights pre-swizzled to [128, 8*128]; all bulk DMAs ride the two
    hardware DGE queues (sync + scalar), first-needed first
  - V_aug [t, d+1] (V plus a ones column that makes the attention accumulator's
    column 128 collect sum(exp)): tiles 0..7 up front, rest as PE filler
  - Q^T, K^T [d, t]: group 0 up front; later groups as PE filler units inside
    earlier groups' score/exp loops (own 1-bank PSUM pool)
  - scores transposed: S^T[s,tq] = matmul(lhsT=K^T[:,s128], rhs=Q^T[:,tq-group])
    over tq groups of 1024, skipping score halves above the diagonal
  - P^T = exp(S^T / sqrt(128)): one ScalarE activation per (s-chunk, group),
    trimmed to the causally valid tq range; no max subtraction (|scores| < ~3)
  - diagonal 128x128 blocks masked on GpSimd (affine_select keeps s <= tq)
  - P^T tiles cached in SBUF; PV accumulates acc[tq,129] over s-chunks in three
    passes of 3/3/2 PSUM accumulators (pass 1 inline with the score loop);
    the leading score/exp iterations of group g+1 are prefetched into group g's
    PV pass phase, where the scalar engine would otherwise idle
  - out = acc[:, :128] * (1 / acc[:, 128]) staged per group, one DMA per pass
"""

import numpy as np
import ml_dtypes

B, T, C, D = 8, 4096, 1024, 128
N_CORES = 8
CT = C // 128          # 8 c-chunks
GQ = 1024              # tq group width
N_G = T // GQ          # 4 tq groups
N_TQ = T // 128        # 32 tq/s tiles of 128
INV_SQRT_D = 1.0 / float(np.sqrt(D))
PASS_TILES = [(0, 1, 2), (3, 4, 5), (6, 7)]   # tq tiles (within group) per PV pass
N_PREFETCH = {1: 2, 2: 5, 3: 12}              # js of group g prefetched into g-1's passes

_CACHE = {}


def _build_nc():
    import concourse.tile as tile
    from concourse import bacc, mybir

    f32 = mybir.dt.float32
    bf16 = mybir.dt.bfloat16

    nc = bacc.Bacc(None, target_bir_lowering=False)
    xt_d = nc.declare_dram_parameter("xt", [CT * N_G, 128, GQ], bf16, isOutput=False)
    wq_d = nc.declare_dram_parameter("wq", [128, CT * D], bf16, isOutput=False)
    wk_d = nc.declare_dram_parameter("wk", [128, CT * D], bf16, isOutput=False)
    wv_d = nc.declare_dram_parameter("wv", [128, CT * D], bf16, isOutput=False)
    out_d = nc.declare_dram_parameter("out", [T, D], f32, isOutput=True)

    with tile.TileContext(nc) as tc:
        with (
            tc.tile_pool(name="consts", bufs=1) as consts,
            tc.tile_pool(name="xt", bufs=1) as xt_pool,
            tc.tile_pool(name="qk", bufs=1) as qk_pool,
            tc.tile_pool(name="vaug", bufs=1) as vaug_pool,
            tc.tile_pool(name="p", bufs=1) as p_pool,
            tc.tile_pool(name="osb", bufs=2) as o_pool,
            tc.tile_pool(name="psS", bufs=2, space="PSUM") as psS,
            tc.tile_pool(name="psO", bufs=3, space="PSUM") as psO,
            tc.tile_pool(name="psQ", bufs=1, space="PSUM") as psQ,
        ):
            wq_sb = consts.tile([128, CT * D], bf16, tag="wq")
            wk_sb = consts.tile([128, CT * D], bf16, tag="wk")
            wv_sb = consts.tile([128, CT * D], bf16, tag="wv")

            # --- DMA issue order: first compute needs wq + wk + xt[:, group 0];
            # wv is not needed until the first V unit (~15us in) ---
            xt_sb = [[None] * N_G for _ in range(CT)]

            def xt_dma(j, m, split=False):
                t_ = xt_pool.tile([128, GQ], bf16, tag=f"xt_{j}_{m}", name=f"xt_{j}_{m}")
                eng = nc.sync if j % 2 == 0 else nc.scalar
                if split:  # land the first half early so compute starts sooner
                    eng.dma_start(t_[:, 0:512], xt_d[j * N_G + m][:, 0:512])
                    eng.dma_start(t_[:, 512:GQ], xt_d[j * N_G + m][:, 512:GQ])
                else:
                    eng.dma_start(t_[:], xt_d[j * N_G + m])
                xt_sb[j][m] = t_

            nc.sync.dma_start(wq_sb[:], wq_d[:])
            nc.scalar.dma_start(wk_sb[:], wk_d[:])
            for j in range(4):
                xt_dma(j, 0, split=(j < 2))
            nc.sync.dma_start(wv_sb[:], wv_d[:])
            for j in range(4, CT):
                xt_dma(j, 0)
            for m in range(1, N_G):
                for j in range(CT):
                    xt_dma(j, m)

            qT = [None] * N_G    # [d=128, GQ] bf16 per tq group
            kT = [None] * N_G
            vaug = [None] * N_TQ  # [s=128, 129] bf16 per s-tile

            def qk_proj_units(g):
                units = []
                for w_sb, dest_list, nm in ((wq_sb, qT, "q"), (wk_sb, kT, "k")):
                    for h in range(2):
                        def unit(g=g, w_sb=w_sb, dest_list=dest_list, nm=nm, h=h):
                            hs = slice(h * 512, (h + 1) * 512)
                            ps = psQ.tile([128, 512], f32, tag="psQ", name=f"ps{nm}_{g}_{h}")
                            for j in range(CT):
                                nc.tensor.matmul(ps[:], w_sb[:, j * D:(j + 1) * D],
                                                 xt_sb[j][g][:, hs],
                                                 start=(j == 0), stop=(j == CT - 1))
                            if dest_list[g] is None:
                                dest_list[g] = qk_pool.tile([128, GQ], bf16,
                                                            tag=f"{nm}_{g}", name=f"{nm}_{g}")
                            nc.vector.tensor_copy(dest_list[g][:, hs], ps[:])
                        units.append(unit)
                return units

            vT = [None] * N_G   # [d=128, GQ] bf16 V^T per group (groups 1..3)

            def vT_unit(g, h):
                """V^T projection for half h of group g (N=512 matmuls), then
                xbar-transpose each 128x128 block into its V_aug tile."""
                def unit(g=g, h=h):
                    hs = slice(h * 512, (h + 1) * 512)
                    ps = psQ.tile([128, 512], f32, tag="psQ", name=f"psvT_{g}_{h}")
                    for j in range(CT):
                        nc.tensor.matmul(ps[:], wv_sb[:, j * D:(j + 1) * D],
                                         xt_sb[j][g][:, hs],
                                         start=(j == 0), stop=(j == CT - 1))
                    if vT[g] is None:
                        vT[g] = qk_pool.tile([128, GQ], bf16, tag=f"vT_{g}",
                                             name=f"vT_{g}")
                    nc.vector.tensor_copy(vT[g][:, hs], ps[:])
                    for k in range(4):
                        i = g * 8 + h * 4 + k
                        v_t = vaug_pool.tile([128, D + 1], bf16, tag=f"v_{i}",
                                             name=f"v_{i}")
                        # all transposes ride the scalar queue; sync stays in
                        # plain-copy xbar mode (mode switches serialize the queue)
                        nc.scalar.dma_start_transpose(
                            v_t[:, 0:D], vT[g][:, (h * 4 + k) * 128:(h * 4 + k + 1) * 128])
                        nc.vector.memset(v_t[:, D:D + 1], 1.0)
                        vaug[i] = v_t
                return unit

            # --- up front: Q/K of group 0, V tiles 0..7 directly (psO free: no
            # accs live yet; direct form avoids transposes during the xt stream) ---
            for u in qk_proj_units(0):
                u()
            for i in range(8):
                off = (i % 8) * 128
                psv = psO.tile([128, D], f32, tag="psO", name=f"psv0_{i}")
                for j in range(CT):
                    nc.tensor.matmul(psv[:], xt_sb[j][0][:, off:off + 128],
                                     wv_sb[:, j * D:(j + 1) * D],
                                     start=(j == 0), stop=(j == CT - 1))
                v_t = vaug_pool.tile([128, D + 1], bf16, tag=f"v_{i}", name=f"v_{i}")
                nc.vector.tensor_copy(v_t[:, 0:D], psv[:])
                nc.vector.memset(v_t[:, D:D + 1], 1.0)
                vaug[i] = v_t

            # score + exp (+ diagonal mask) for one (g, js); returns the P tile
            def score_exp(g, js):
                off = max(0, (js - 8 * g)) * 128
                pss = psS.tile([128, GQ], f32, tag="psS", name=f"pss_{g}_{js}")
                for h in range(2):
                    if (h + 1) * 512 > off:
                        nc.tensor.matmul(pss[:, h * 512:(h + 1) * 512],
                                         kT[js // 8][:, (js % 8) * 128:(js % 8 + 1) * 128],
                                         qT[g][:, h * 512:(h + 1) * 512],
                                         start=True, stop=True)
                # js < 12 tags are double-buffered: their prefetch (next group)
                # overlaps the previous group's PV passes reading the old slot
                p_t = p_pool.tile([128, GQ], bf16, tag=f"p_{js}",
                                  bufs=(2 if js < 12 else 1), name=f"p_{g}_{js}")
                nc.scalar.activation(p_t[:, off:GQ], pss[:, off:GQ],
                                     mybir.ActivationFunctionType.Exp,
                                     scale=INV_SQRT_D)
                if js >= 8 * g:
                    nc.gpsimd.affine_select(
                        out=p_t[:, off:off + 128],
                        in_=p_t[:, off:off + 128],
                        compare_op=mybir.AluOpType.is_ge,
                        fill=0.0,
                        base=0,
                        pattern=[[1, 128]],
                        channel_multiplier=-1,
                    )
                return p_t

            filler_sched = {
                0: list(qk_proj_units(1)) + [vT_unit(1, 0)],
                1: [vT_unit(1, 1)] + list(qk_proj_units(2)) + [vT_unit(2, 0)],
                2: [vT_unit(2, 1)] + list(qk_proj_units(3)) + [vT_unit(3, 0)],
                3: [vT_unit(3, 1)],
            }

            def finalize(g, tiles, accs, o_stage):
                for t in tiles:
                    recip = o_pool.tile([128, 1], f32, tag="recip",
                                        name=f"recip_{g}_{t}")
                    nc.vector.reciprocal(recip[:], accs[t][:, D:D + 1])
                    nc.vector.tensor_scalar_mul(o_stage[:, t * D:(t + 1) * D],
                                                accs[t][:, 0:D], recip[:])
                row0 = g * GQ + tiles[0] * 128
                n_t = len(tiles)
                nc.sync.dma_start(
                    out_d[row0:row0 + n_t * 128, :].rearrange("(t p) d -> p t d", p=128),
                    o_stage[:, tiles[0] * D:(tiles[0] + n_t) * D].rearrange(
                        "p (t d) -> p t d", t=n_t),
                )

            prefetched = {}   # g -> list of P tiles for js 0..n_pre-1
            for g in range(N_G):
                n_js = 8 * g + 8
                n_pre = N_PREFETCH.get(g, 0)
                fillers = list(filler_sched[g])
                p_cache = [None] * n_js
                o_stage = o_pool.tile([128, 8 * D], f32, tag="o8", name=f"o_{g}")

                accs1 = {}
                for t in PASS_TILES[0]:
                    accs1[t] = psO.tile([128, D + 1], f32, tag="psO",
                                        name=f"acc1_{g}_{t}")
                # get the scalar engine going on fresh score/exp work before the
                # PE chews through the prefetched-js PV block
                if n_pre < n_js:
                    p_cache[n_pre] = score_exp(g, n_pre)
                # PV for the prefetched js (score/exp already done last group)
                for js in range(n_pre):
                    p_cache[js] = prefetched[g][js]
                    for t in PASS_TILES[0]:
                        nc.tensor.matmul(accs1[t][:],
                                         p_cache[js][:, t * 128:(t + 1) * 128],
                                         vaug[js][:],
                                         start=(js == 0), stop=False)
                fill_every = max(1, (n_js - n_pre) // (len(fillers) + 1)) if fillers else 0
                for js in range(n_pre, n_js):
                    p_t = p_cache[js] if p_cache[js] is not None else score_exp(g, js)
                    p_cache[js] = p_t
                    for t in PASS_TILES[0]:
                        tq_tile = 8 * g + t
                        if js <= tq_tile:
                            nc.tensor.matmul(accs1[t][:],
                                             p_t[:, t * 128:(t + 1) * 128],
                                             vaug[js][:],
                                             start=(js == 0), stop=(js == tq_tile))
                    if fillers and (js - n_pre + 1) % fill_every == 0:
                        fillers.pop(0)()
                for u in fillers:
                    u()
                finalize(g, PASS_TILES[0], accs1, o_stage)

                # PV passes 2/3, with next group's leading score/exp interleaved
                pre_next = []
                n_pre_next = N_PREFETCH.get(g + 1, 0)
                for pi, tiles in enumerate(PASS_TILES[1:]):
                    accs = {}
                    for t in tiles:
                        accs[t] = psO.tile([128, D + 1], f32, tag="psO",
                                           name=f"acc_{g}_{t}")
                    for js in range(n_js):
                        for t in tiles:
                            tq_tile = 8 * g + t
                            if js <= tq_tile:
                                nc.tensor.matmul(accs[t][:],
                                                 p_cache[js][:, t * 128:(t + 1) * 128],
                                                 vaug[js][:],
                                                 start=(js == 0), stop=(js == tq_tile))
                        # interleave next group's score/exp prefetch
                        if len(pre_next) < n_pre_next and js % 3 == 2:
                            pre_next.append(score_exp(g + 1, len(pre_next)))
                    finalize(g, tiles, accs, o_stage)
                while len(pre_next) < n_pre_next:
                    pre_next.append(score_exp(g + 1, len(pre_next)))
                prefetched[g + 1] = pre_next

    nc.compile()
    return nc


def _get_nc():
    if "nc" not in _CACHE:
        _CACHE["nc"] = _build_nc()
    return _CACHE["nc"]


def _pack_xt(xb):
    """x[b] [T, C] f32 -> [CT*N_G, 128, GQ] bf16 tiles of x^T."""
    xt = np.ascontiguousarray(xb.T).astype(ml_dtypes.bfloat16)  # [C, T]
    xt = xt.reshape(CT, 128, N_G, GQ).transpose(0, 2, 1, 3)     # [j, m, 128, GQ]
    return np.ascontiguousarray(xt.reshape(CT * N_G, 128, GQ))


def _pack_w(w):
    """W [C, D] f32 -> [128, CT*D] bf16: chunk j of rows -> columns j*D:(j+1)*D."""
    wb = w.astype(ml_dtypes.bfloat16).reshape(CT, 128, D).transpose(1, 0, 2)
    return np.ascontiguousarray(wb.reshape(128, CT * D))


def kernel(x, Wq, Wk, Wv):
    from concourse.bass_utils import run_bass_kernel_spmd

    nc = _get_nc()
    wq, wk, wv = _pack_w(Wq), _pack_w(Wk), _pack_w(Wv)
    in_maps = []
    for b in range(N_CORES):
        in_maps.append({"xt": _pack_xt(x[b]), "wq": wq, "wk": wk, "wv": wv})
    res = run_bass_kernel_spmd(nc, in_maps, core_ids=list(range(N_CORES)))
    return np.stack([res.results[b]["out"] for b in range(N_CORES)], axis=0)
